# revision 15
# baseline (speedup 1.0000x reference)
"""ArDCA forward kernel for 8 trn2 NeuronCores.

z[m,i,a] = h[i,a] + sum_{j<i} sum_b J[i,j,b,a] * X[m,j,b]

Flattening (j,b)->K and (i,a)->N, this is one block-upper-triangular matmul
Z^T = Jmat^T @ X^T where J[i].reshape(L*Q, Q) is natively the i-th column
block of the stationary operand (no transpose of J needed).

Sharding: the 5376 output columns (i,a) are cut into 42 column-tiles of 128,
distributed over the 8 cores into 6 uniform slots per core (SPMD runs an
identical graph on every core; which column-tile a slot computes is decided
purely by the host-packed per-core J/h data — a slot whose tile needs fewer
K-tiles than the slot budget just gets zero-padded J). Each slot is one PSUM
accumulation chain: matmul(psum, lhsT=J_tile(128x128) bf16, rhs=XT_tile
(128x512) fp8) with f32 accumulation; a DVE tensor_scalar add of h evacuates
PSUM -> SBUF; the result is DMA'd out in f16 (host upcasts). X^T (fp8: exact)
is resident in SBUF; J streams. All DRAM buffers are host-packed
partition-major so DMA descriptor runs per partition are >=512B.
"""

import math
import numpy as np
import ml_dtypes

M, L, Q = 512, 256, 21
LQ = L * Q                      # 5376 = 42*128
COLS = 128                      # output columns per group (column-tile)
NG = LQ // COLS                 # 42 column-tiles
NCORES = 8
NXT = LQ // 128                 # 42 X k-tiles
CKJ = 12                        # J k-tiles per DMA chunk
BF16 = ml_dtypes.bfloat16
FP8 = ml_dtypes.float8_e4m3
# Hybrid precision: the first F k-tiles of each chain are fp8e4m3 and run as
# DoubleRow pairs (2 k-tiles per PE instruction, ~1.7x), the rest stay bf16.
# All of J is pre-scaled by JS=1024 (exact for bf16, centers fp8 range); one
# psum*(1/JS)+h rescale at evacuation undoes it. Host-measured rel err of
# this split vs f32 reference: 1.816e-2 (threshold 2e-2; HW matched the
# host model to ~5 digits at the 38- and 46-ktile splits).
JS = 1024.0
F_BY_BUDGET = {2: 0, 10: 4, 18: 8, 26: 10, 34: 14, 42: 16}


def _ktiles(g: int) -> int:
    i_max = (COLS * g + COLS - 1) // Q      # highest i in the tile
    return max(1, math.ceil(Q * i_max / 128))


def _plan():
    """Uniform slot structure + serpentine group->core assignment."""
    items = sorted(range(NG), key=lambda g: (-_ktiles(g), g))
    nslots = math.ceil(NG / NCORES)                      # 6
    budgets = [_ktiles(items[NCORES * r]) for r in range(nslots)]
    assign = [[None] * nslots for _ in range(NCORES)]    # assign[core][slot] = group
    for r in range(nslots):
        row = items[NCORES * r: NCORES * (r + 1)]
        for k, g in enumerate(row):
            core = k if r % 2 == 0 else NCORES - 1 - k
            assign[core][r] = g
    offs = [COLS * sum(budgets[:r]) for r in range(nslots)]  # jp col offset per slot
    return budgets, assign, offs


BUDGETS, ASSIGN, JOFFS = _plan()
S = len(BUDGETS)                 # 6 slots per core
FS = [F_BY_BUDGET[b] for b in BUDGETS]          # fp8 k-tiles per slot
J8OFFS = [COLS * sum(FS[:r]) for r in range(S)]
J16OFFS = [COLS * sum(BUDGETS[q] - FS[q] for q in range(r)) for r in range(S)]
W8 = COLS * sum(FS)              # jp8 total columns per core
WJ = COLS * sum(BUDGETS[r] - FS[r] for r in range(S))   # jp16 cols per core
WX = NXT * M                     # xt total columns (21504)
# ascending budgets: slot k first-touches only X tiles [B_{k-1}, B_k), so the
# X demand spreads over the whole run and a single consumption-order DMA
# stream (split over both HWDGE rings) can stay ahead of the PE
SLOT_ORDER = sorted(range(S), key=lambda r: BUDGETS[r])
# X chunk k = the tiles slot k first-touches
XCHUNKS = tuple(
    BUDGETS[SLOT_ORDER[k]] - (BUDGETS[SLOT_ORDER[k - 1]] if k else 0)
    for k in range(S)
)


def _build_nc():
    import concourse.bacc as bacc
    import concourse.mybir as mybir
    from concourse import tile

    f32 = mybir.dt.float32
    bf16 = mybir.dt.bfloat16
    fp8 = mybir.dt.float8e4

    nc = bacc.Bacc(None, target_bir_lowering=False, debug=False)
    xt_ext = nc.declare_dram_parameter("xt", [128, WX], fp8, isOutput=False)
    jp_ext = nc.declare_dram_parameter("jp", [128, WJ], bf16, isOutput=False)
    jp8_ext = nc.declare_dram_parameter("jp8", [128, W8], fp8, isOutput=False)
    hb_ext = nc.declare_dram_parameter("hb", [COLS, S], f32, isOutput=False)
    f16 = mybir.dt.float16
    out_ext = nc.declare_dram_parameter("out", [S * COLS, M], f16, isOutput=True)

    with tile.TileContext(nc) as tc:
        with (
            tc.tile_pool(name="x", bufs=1) as xpool,
            tc.tile_pool(name="j", bufs=1) as jpool,
            tc.tile_pool(name="ps", bufs=5, space="PSUM") as ppool,
            tc.tile_pool(name="psf", bufs=1, space="PSUM") as pfpool,
            tc.tile_pool(name="o", bufs=6) as opool,
            tc.tile_pool(name="c", bufs=1) as cpool,
        ):

            # HAM warm-up: the PE clock-gate releases (1.2 -> 2.4 GHz) only
            # after ~3.4us of sustained matmul activity. The first HWDGE data
            # lands only ~11us in (issue ~6.7us + ring latency), so the
            # warm-up dummies must bridge that whole window to keep the HAM
            # ramp alive; fewer dummies just idles the PE and delays the
            # 2.4GHz transition (measured +3us on the stream).
            NWARM = 13
            zw = cpool.tile([128, 128], bf16, tag="zw")
            nc.vector.memset(zw[:], 0.0)
            # rhs for the dummies: the same 128 zero columns read 4x via a
            # zero-stride AP dim -> free size 512 with only a 32KB memset
            import concourse.bass as _bass
            _a = zw[:]
            zw_rhs = _bass.AP(_a.tensor, _a.offset,
                              [_a.ap[0], (0, M // 128), _a.ap[1]])
            hb_t = cpool.tile([COLS, S], f32, tag="hb")
            nc.gpsimd.dma_start(out=hb_t[:], in_=hb_ext[:])

            # one global DMA stream in exact consumption order, split over the
            # two HWDGE rings greedily by queued bytes (each ring is FIFO, so
            # balanced byte loads keep arrival order ~= consumption order);
            # every tile unique-tagged and resident (no pool-recycle waits)
            rings = [nc.sync, nc.scalar]
            ring_bytes = [0, 0]

            def ring_dma(out_ap, in_ap, nbytes):
                i = 0 if ring_bytes[0] <= ring_bytes[1] else 1
                rings[i].dma_start(out=out_ap, in_=in_ap)
                ring_bytes[i] += nbytes

            xts = []           # (tile, local_ktile) per global X ktile
            jtiles = {}        # (slot, chunk_start) -> tile
            xoff = 0

            def dr_matmul(ps_ap, j8, p, c0, c1, start, stop):
                # DoubleRow: one PE instruction contracts k-tiles 2p and 2p+1.
                # lhsT [128, 2, COLS]: two adjacent 128x128 fp8 J tiles;
                # rhs  [128, 2, c1-c0]: the matching X k-tile pair (stride M
                # between the tiles inside one resident X item).
                a = j8[:, 2 * p * COLS:(2 * p + 2) * COLS]
                lhsT = _bass.AP(a.tensor, a.offset,
                                [a.ap[0], (COLS, 2), (1, COLS)])
                xt_t, xl = xts[2 * p]
                xt_t2, xl2 = xts[2 * p + 1]
                assert xt_t is xt_t2 and xl2 == xl + 1, "X pair split across items"
                b = xt_t[:, xl * M + c0:xl * M + c1]
                rhs = _bass.AP(b.tensor, b.offset,
                               [b.ap[0], (M, 2), (1, c1 - c0)])
                nc.tensor.matmul(
                    ps_ap, lhsT, rhs, start=start, stop=stop,
                    perf_mode=mybir.MatmulPerfMode.DoubleRow,
                )

            def emit_x(si, cx):
                nonlocal xoff
                # split a slot's fresh X window into <=4-tile items so arrival
                # is incremental
                done = 0
                while done < cx:
                    n = min(4, cx - done)
                    xt_t = xpool.tile([128, n * M], fp8, tag=f"x{si}_{done}")
                    ring_dma(xt_t[:], xt_ext[:, xoff * M:(xoff + n) * M],
                             n * M * 128)
                    for t in range(n):
                        xts.append((xt_t, t))
                    xoff += n
                    done += n

            def jchunks(T):
                cs, t = [], 0
                while t < T:
                    ck = min(CKJ, T - t)
                    cs.append((t, ck))
                    t += ck
                return cs

            j8tiles = {}
            for si, r in enumerate(SLOT_ORDER):
                T, F = BUDGETS[r], FS[r]
                cs = jchunks(T - F)
                # slot 0 touches its X tiles immediately; later slots touch
                # their fresh X window only from chain position B_{k-1}, i.e.
                # after their first J chunk is already being consumed
                if si == 0:
                    emit_x(si, XCHUNKS[si])
                if F:
                    j8 = jpool.tile([128, F * COLS], fp8, tag=f"j8_{r}")
                    c8 = J8OFFS[r]
                    ring_dma(j8[:], jp8_ext[:, c8:c8 + F * COLS], F * COLS * 128)
                    j8tiles[r] = j8
                    if si > 0:
                        emit_x(si, XCHUNKS[si])
                for idx, (t, ck) in enumerate(cs):
                    jt = jpool.tile([128, ck * COLS], bf16, tag=f"j{r}_{t}")
                    c0 = J16OFFS[r] + t * COLS
                    ring_dma(jt[:], jp_ext[:, c0:c0 + ck * COLS], ck * COLS * 256)
                    jtiles[(r, t)] = jt
                    if si > 0 and idx == 0 and not F:
                        emit_x(si, XCHUNKS[si])


            stores = []
            for si, r in enumerate(SLOT_ORDER):
                T = BUDGETS[r]
                if si == S - 1:
                    # final slot: split the chain by samples (N=448 then
                    # N=64, same J tiles). The wide chain's evac+store
                    # overlap the narrow chain's matmuls, so only the tiny
                    # N=64 evac+store is exposed after the last matmul.
                    HA = 448
                    ps_a = pfpool.tile([COLS, HA], f32, tag="psA")
                    ps_b = pfpool.tile([COLS, M - HA], f32, tag="psB")
                    ot = opool.tile([COLS, M], f16, tag="ot")
                    rows = slice(r * COLS, (r + 1) * COLS)
                    Ff = FS[r]
                    for part, (ps_p, c0, c1, ring) in enumerate(
                        [(ps_a, 0, HA, nc.sync),
                         (ps_b, HA, M, nc.scalar)]
                    ):
                        for p in range(Ff // 2):
                            dr_matmul(ps_p[:], j8tiles[r], p, c0, c1,
                                      start=(p == 0), stop=False)
                        for t, ck in jchunks(T - Ff):
                            jt = jtiles[(r, t)]
                            for tl in range(ck):
                                tt = Ff + t + tl
                                xt_t, xl = xts[tt]
                                nc.tensor.matmul(
                                    ps_p[:],
                                    jt[:, tl * COLS:(tl + 1) * COLS],
                                    xt_t[:, xl * M + c0:xl * M + c1],
                                    start=(tt == 0),
                                    stop=(tt == T - 1),
                                )
                        if part == 0:
                            # flush the deferred slot stores now: their evacs
                            # completed long ago, so these issue immediately
                            # and queue behind the remaining ring loads
                            for k, (dst, src) in enumerate(stores):
                                rings[k % 2].dma_start(out=dst, in_=src)
                            stores = []
                        nc.vector.tensor_scalar(
                            ot[:, c0:c1], ps_p[:], 1.0 / JS, hb_t[:, r:r + 1],
                            mybir.AluOpType.mult, mybir.AluOpType.add)
                        ring.dma_start(out=out_ext[rows, c0:c1],
                                       in_=ot[:, c0:c1])
                    continue
                ps = ppool.tile([COLS, M], f32, tag="ps")
                F = FS[r]
                if si == 0:
                    for w in range(NWARM):
                        nc.tensor.matmul(
                            ps[:], zw[:], zw_rhs,
                            start=(w == 0), stop=False,
                        )
                for p in range(F // 2):
                    dr_matmul(ps[:], j8tiles[r], p, 0, M,
                              start=(p == 0 and si != 0), stop=False)
                for t, ck in jchunks(T - F):
                    jt = jtiles[(r, t)]
                    for tl in range(ck):
                        tt = F + t + tl
                        xt_t, xl = xts[tt]
                        nc.tensor.matmul(
                            ps[:],
                            jt[:, tl * COLS:(tl + 1) * COLS],
                            xt_t[:, xl * M:(xl + 1) * M],
                            start=(tt == 0 and si != 0),
                            stop=(tt == T - 1),
                        )
                ot = opool.tile([COLS, M], f16, tag="ot")
                # f16 out tile (0.05% quantization, halves store bytes);
                # store deferred to the HWDGE rings after all loads so the
                # SWDGE queue stays empty (cheaper end-of-kernel drain)
                nc.vector.tensor_scalar(
                    ot[:], ps[:], 1.0 / JS, hb_t[:, r:r + 1],
                    mybir.AluOpType.mult, mybir.AluOpType.add)
                stores.append((out_ext[r * COLS:(r + 1) * COLS, :], ot[:]))

    nc.finalize()
    return nc


_CACHE = {}


def _get_nc():
    if "nc" not in _CACHE:
        _CACHE["nc"] = _build_nc()
    return _CACHE["nc"]


def _pack_inputs(X_oh, h_pos, J):
    """Build per-core in_maps (host-side shard + layout)."""
    XT = np.ascontiguousarray(X_oh.transpose(1, 2, 0).reshape(LQ, M))
    xt = np.ascontiguousarray(
        XT.reshape(NXT, 128, M).transpose(1, 0, 2).reshape(128, WX)
    ).astype(FP8)

    JT = (J.reshape(L, LQ, Q) * np.float32(JS)).astype(np.float32)
    h32 = h_pos.astype(np.float32)

    in_maps = []
    for core in range(NCORES):
        jp = np.zeros((128, WJ), dtype=BF16)
        jp8 = np.zeros((128, W8), dtype=FP8)
        hb = np.zeros((COLS, S), dtype=np.float32)
        for r in range(S):
            g = ASSIGN[core][r]
            if g is None:
                continue
            T, F = BUDGETS[r], FS[r]
            blk = np.zeros((T * 128, COLS), dtype=np.float32)
            # columns are global output indices ia = COLS*g + col, i = ia//Q
            ia0 = COLS * g
            col = 0
            while col < COLS:
                i, a0 = divmod(ia0 + col, Q)
                na = min(Q - a0, COLS - col)        # run of columns within one i
                rows = Q * i                        # strictly-lower mask: j < i
                blk[:rows, col:col + na] = JT[i][:rows, a0:a0 + na]
                hb[col:col + na, r] = h32[i, a0:a0 + na]
                col += na
            if F:
                jp8[:, J8OFFS[r]:J8OFFS[r] + F * COLS] = (
                    blk[:F * 128].reshape(F, 128, COLS)
                    .transpose(1, 0, 2).reshape(128, F * COLS).astype(FP8)
                )
            jp[:, J16OFFS[r]:J16OFFS[r] + (T - F) * COLS] = (
                blk[F * 128:].reshape(T - F, 128, COLS)
                .transpose(1, 0, 2).reshape(128, (T - F) * COLS).astype(BF16)
            )
        in_maps.append({"xt": xt, "jp": jp, "jp8": jp8, "hb": hb})
    return in_maps


def _unpack_outputs(results):
    outT = np.zeros((LQ, M), dtype=np.float32)
    for core in range(NCORES):
        o = results[core]["out"]
        for r in range(S):
            g = ASSIGN[core][r]
            if g is None:
                continue
            outT[COLS * g:COLS * (g + 1)] = o[r * COLS:(r + 1) * COLS]
    return np.ascontiguousarray(outT.reshape(L, Q, M).transpose(2, 0, 1))


def _run(in_maps, trace=False, **kw):
    from concourse.bass_utils import run_bass_kernel_spmd

    nc = _get_nc()
    return run_bass_kernel_spmd(nc, in_maps, list(range(NCORES)), trace=trace, **kw)


def kernel(X_oh, h_pos, J):
    X_oh = np.asarray(X_oh, dtype=np.float32)
    h_pos = np.asarray(h_pos, dtype=np.float32)
    J = np.asarray(J, dtype=np.float32)
    in_maps = _pack_inputs(X_oh, h_pos, J)
    res = _run(in_maps)
    return _unpack_outputs(res.results)

